# revision 1
# baseline (speedup 1.0000x reference)
"""Trainium2 Bass kernel for nn_Attention (dense transformer block:
QKV proj + RoPE + causal GQA attention + o_proj), SPMD over 8 NeuronCores.

Sharding: core c -> (batch b = c//4, head-group g = c%4). Each core computes
4 query heads + its kv head for one batch, then the head outputs are
AllGather'd within the 4-core batch group and each core computes a disjoint
512-column slice of the o_proj output.

All matmuls run as float32r (TF32-like, ~4x faster than plain fp32 on the
PE, measured rel err ~1.5e-4). Host-side work is only slicing/transposition
(layout prep) and the final concatenation of per-core output slices.
"""

import sys
import time

sys.path.insert(0, "/opt/trn_rl_repo")

import numpy as np

import concourse.bass as bass
import concourse.mybir as mybir
import concourse.tile as tile
from concourse import bacc
from concourse.masks import make_identity

F32 = mybir.dt.float32
F32R = mybir.dt.float32r
P = 128
HD = 128            # head dim
NHL = 4             # query heads per core
E = 2048            # hidden
DQ = NHL * HD       # 512, local q-projection width / o-slice width
SCALE = 1.0 / np.sqrt(np.float32(HD))
REPLICA_GROUPS = [[0, 1, 2, 3], [4, 5, 6, 7]]
NO_COLLECTIVE = False  # replace AllGather with a local DMA (timeline-sim only)
GPSIMD_MASK = False    # causal mask via gpsimd.affine_select instead of DVE mult
AG_HALVES = 2          # AllGathers per head (1, 2, or 4; must divide NQC)
X_ROWS = False         # load x as contiguous [128, E] row tiles instead of blocks
DEN_PE = True          # softmax denominator via PE ones-matmul (else DVE adds)


def r32(ap):
    return ap.bitcast(F32R)


def build_program(S=2048, reps=1, n_cores=8):
    """Build the per-core SPMD Bass program. Returns compiled nc."""
    ST = S // P          # 128-row tiles along sequence
    NQC = S // 512       # 512-wide chunks along sequence
    ET = E // P          # 16 tiles along hidden

    nc = bacc.Bacc("TRN2", target_bir_lowering=False, debug=False,
                   num_devices=n_cores)

    x_in = nc.declare_dram_parameter("x", [S, E], F32, isOutput=False)
    wqT_in = nc.declare_dram_parameter("wqT", [E, DQ], F32, isOutput=False)
    wkT_in = nc.declare_dram_parameter("wkT", [E, HD], F32, isOutput=False)
    wvT_in = nc.declare_dram_parameter("wvT", [E, HD], F32, isOutput=False)
    woT_in = nc.declare_dram_parameter("woT", [E, DQ], F32, isOutput=False)
    cosT_in = nc.declare_dram_parameter("cosT", [HD, S], F32, isOutput=False)
    sinT_in = nc.declare_dram_parameter("sinT", [HD, S], F32, isOutput=False)
    out_d = nc.declare_dram_parameter("out", [DQ, S], F32, isOutput=True)

    with tile.TileContext(nc) as tc:
        with nc.allow_low_precision(reason="float32r rounding for PE operands"):
            _emit(tc, nc, S, ST, NQC, ET, reps,
                  x_in, wqT_in, wkT_in, wvT_in, woT_in, cosT_in, sinT_in, out_d)

    nc.compile()
    return nc


def _emit(tc, nc, S, ST, NQC, ET, reps,
          x_in, wqT_in, wkT_in, wvT_in, woT_in, cosT_in, sinT_in, out_d):
    from contextlib import ExitStack

    ctx = ExitStack()
    with ctx:
        const = ctx.enter_context(tc.tile_pool(name="const", bufs=1))
        wpool = ctx.enter_context(tc.tile_pool(name="wpool", bufs=1))
        qkv = ctx.enter_context(tc.tile_pool(name="qkv", bufs=1))
        dram = ctx.enter_context(tc.tile_pool(name="dram", bufs=1, space="DRAM"))

        # ---- constants ----
        ident = const.tile([P, P], F32)
        make_identity(nc, ident[:])
        masks = const.tile([P, 4 * 512], F32)
        nc.gpsimd.memset(masks[:], 1.0)
        for t in range(4):
            # valid(k_local, q_local) = (q_local - k_local - 128*t) >= 0
            nc.gpsimd.affine_select(
                out=masks[:, t * 512:(t + 1) * 512],
                in_=masks[:, t * 512:(t + 1) * 512],
                compare_op=mybir.AluOpType.is_ge,
                fill=0.0, base=-P * t, pattern=[[1, 512]],
                channel_multiplier=-1,
            )
        ones_stage = const.tile([P, P], F32)
        nc.gpsimd.memset(ones_stage[:], 1.0)
        ones_red = const.tile([P, 1], F32R)
        nc.vector.tensor_copy(ones_red[:], ones_stage[:, 0:1])
        ones_col = const.tile([1, P], F32R)
        nc.vector.tensor_copy(ones_col[:], ones_stage[0:1, :])

        # ---- persistent SBUF ----
        QT_sb = qkv.tile([P, NHL, S], F32R)
        KT_sb = qkv.tile([P, S], F32R)
        V_sb = qkv.tile([P, ST, HD], F32R)

        # collective bounce buffers (DRAM), one per (head, seq-half)
        NHALF = min(AG_HALVES, NQC)
        SH = S // NHALF
        agin = [[dram.tile([P, SH], F32, name=f"agin{h}_{hf}")
                 for hf in range(NHALF)] for h in range(NHL)]
        agout = [[dram.tile([4 * P, SH], F32, name=f"agout{h}_{hf}")
                  for hf in range(NHALF)] for h in range(NHL)]

        for rep in range(reps):
            _emit_rep(tc, nc, S, ST, NQC, ET, ctx, rep,
                      x_in, wqT_in, wkT_in, wvT_in, woT_in, cosT_in, sinT_in,
                      out_d, ident, masks, ones_red, ones_col,
                      QT_sb, KT_sb, V_sb, agin, agout)


def _emit_oproj(nc, h, NQC, cph, NHALF, agout, af_pool, oo_ps, woT_sb,
                outAcc, out_d):
    out_r = out_d.rearrange("(ot p) s -> p ot s", p=P)
    for hf in range(NHALF):
        ag_r = agout[h][hf].rearrange("(mt p) s -> p mt s", p=P)
        for sch in range(cph):
            sc = hf * cph + sch
            s0 = sc * 512
            af = af_pool.tile([P, 4, 512], F32R, name="af", tag="af")
            nc.gpsimd.dma_start(
                af[:], ag_r[:, :, sch * 512:(sch + 1) * 512].bitcast(F32R))
            for ot in range(4):
                po = oo_ps.tile([P, 512], F32, name="po", tag="po")
                for mt in range(4):
                    nc.tensor.matmul(
                        po[:],
                        woT_sb[:, 4 * mt + h, ot * P:(ot + 1) * P],
                        af[:, mt, :],
                        start=(mt == 0), stop=(mt == 3))
                acc = outAcc[:, ot, s0:s0 + 512]
                if h == 0:
                    if ot % 2 == 0:
                        nc.scalar.copy(acc, po[:])
                    else:
                        nc.vector.tensor_copy(acc, po[:])
                else:
                    nc.vector.tensor_add(acc, acc, po[:])
            if h == NHL - 1:
                nc.sync.dma_start(out_r[:, :, s0:s0 + 512],
                                  outAcc[:, :, s0:s0 + 512])


def _emit_rep(tc, nc, S, ST, NQC, ET, ctx, rep,
              x_in, wqT_in, wkT_in, wvT_in, woT_in, cosT_in, sinT_in,
              out_d, ident, masks, ones_red, ones_col,
              QT_sb, KT_sb, V_sb, agin, agout):
    from contextlib import ExitStack


    # ================= projection phase =================
    with ExitStack() as pctx:
        xn_pool = pctx.enter_context(tc.tile_pool(name="xn", bufs=8))
        trig_pool = pctx.enter_context(tc.tile_pool(name="trig", bufs=1))
        xt_pool = pctx.enter_context(tc.tile_pool(name="xt", bufs=18))
        rope_pool = pctx.enter_context(tc.tile_pool(name="rope", bufs=2))
        vt_pool = pctx.enter_context(tc.tile_pool(name="vt", bufs=2))
        pt_ps = pctx.enter_context(tc.tile_pool(name="pt_ps", bufs=3, space="PSUM"))
        pj_ps = pctx.enter_context(tc.tile_pool(name="pj_ps", bufs=4, space="PSUM"))

        cosT_sb = trig_pool.tile([P, S], F32)
        sinT_sb = trig_pool.tile([P, S], F32)
        wqT_sb = trig_pool.tile([P, ET, DQ], F32R)
        wkT_sb = trig_pool.tile([P, ET, HD], F32R)
        wvT_sb = trig_pool.tile([P, ET, HD], F32R)

        x_r = x_in.rearrange("(sq st4 p) e -> p sq st4 e", p=P, st4=4)
        wq_r = wqT_in.rearrange("(et p) d -> p et d", p=P).bitcast(F32R)
        wk_r = wkT_in.rearrange("(et p) d -> p et d", p=P).bitcast(F32R)
        wv_r = wvT_in.rearrange("(et p) d -> p et d", p=P).bitcast(F32R)

        for sc in range(NQC):
            s0 = sc * 512
            # transpose all 16 e-tiles of this s-chunk first
            xrows = []
            if X_ROWS:
                for st4 in range(4):
                    xr = xn_pool.tile([P, E], F32, name=f"xr{st4}",
                                      tag=f"xr{st4}", bufs=1)
                    nc.sync.dma_start(
                        xr[:], x_in[s0 + st4 * P:s0 + (st4 + 1) * P, :])
                    xrows.append(xr)
            xts = []
            for et in range(ET):
                if X_ROWS:
                    xnb = None
                else:
                    xnb = xn_pool.tile([P, 4, P], F32, name="xnb", tag="xnb")
                    nc.sync.dma_start(
                        xnb[:], x_r[:, sc, :, et * P:(et + 1) * P])
                if sc == 0:
                    # interleave weight-slice DMAs with the first chunk's
                    # x loads so the first matmuls aren't starved
                    nc.sync.dma_start(wqT_sb[:, et, :], wq_r[:, et, :])
                    nc.sync.dma_start(wkT_sb[:, et, :], wk_r[:, et, :])
                    nc.sync.dma_start(wvT_sb[:, et, :], wv_r[:, et, :])
                ptile = pt_ps.tile([P, 512], F32, name="ptile", tag="ptile")
                for st4 in range(4):
                    src = (xrows[st4][:, et * P:(et + 1) * P] if X_ROWS
                           else xnb[:, st4, :])
                    nc.tensor.transpose(
                        ptile[:, st4 * P:(st4 + 1) * P], src, ident[:])
                xt_t = xt_pool.tile([P, 512], F32R, name="xts", tag="xts")
                nc.scalar.copy(xt_t[:], ptile[:])
                xts.append(xt_t)
            if sc == 0:
                nc.sync.dma_start(cosT_sb[:], cosT_in[:])
                nc.sync.dma_start(sinT_sb[:], sinT_in[:])

            # d6-outer matmul loop over resident xts tiles
            cos_c = cosT_sb[:, s0:s0 + 512]
            sin_c = sinT_sb[:, s0:s0 + 512]
            for d6 in range(6):
                pp = pj_ps.tile([P, 512], F32, name="pp", tag="pp")
                for et in range(ET):
                    if d6 < 4:
                        lhsT = wqT_sb[:, et, d6 * HD:(d6 + 1) * HD]
                    elif d6 == 4:
                        lhsT = wkT_sb[:, et, :]
                    else:
                        lhsT = wvT_sb[:, et, :]
                    nc.tensor.matmul(pp[:], lhsT, xts[et][:],
                                     start=(et == 0), stop=(et == ET - 1))
                if d6 < 5:
                    dst = (QT_sb[:, d6, s0:s0 + 512] if d6 < 4
                           else KT_sb[:, s0:s0 + 512])
                    t1 = rope_pool.tile([P, 512], F32, name="t1", tag="t1")
                    t2 = rope_pool.tile([P, 512], F32, name="t2", tag="t2")
                    nc.vector.tensor_tensor(t1[:], pp[:], cos_c,
                                            mybir.AluOpType.mult)
                    # sinT arrives with rows 0:64 pre-negated (host side)
                    nc.vector.tensor_tensor(t2[0:64, :], pp[64:128, :],
                                            sin_c[0:64, :],
                                            mybir.AluOpType.mult)
                    nc.vector.tensor_tensor(t2[64:128, :], pp[0:64, :],
                                            sin_c[64:128, :],
                                            mybir.AluOpType.mult)
                    nc.vector.tensor_tensor(dst[:], t1[:], t2[:],
                                            mybir.AluOpType.add)
                else:
                    vts = vt_pool.tile([P, 512], F32, name="vts", tag="vts")
                    nc.scalar.copy(vts[:], pp[:])
                    for st4 in range(4):
                        pv_t = pt_ps.tile([P, 512], F32, name="pvt",
                                          tag="ptile")[:, 0:P]
                        nc.tensor.transpose(pv_t[:],
                                            vts[:, st4 * P:(st4 + 1) * P],
                                            ident[:])
                        nc.scalar.copy(V_sb[:, sc * 4 + st4, :], pv_t[:])

    # ================= attention + o_proj phase =================
    with ExitStack() as actx:
        ex_pool = actx.enter_context(tc.tile_pool(name="ex", bufs=8))
        dn_pool = actx.enter_context(tc.tile_pool(name="dn", bufs=2))
        sm_pool = actx.enter_context(tc.tile_pool(name="sm", bufs=2))
        bc_pool = actx.enter_context(tc.tile_pool(name="bc", bufs=2))
        oh_pool = actx.enter_context(tc.tile_pool(name="oh", bufs=3))
        af_pool = actx.enter_context(tc.tile_pool(name="af", bufs=2))
        oo_pool = actx.enter_context(tc.tile_pool(name="oo", bufs=2))
        sc_ps = actx.enter_context(tc.tile_pool(name="sc_ps", bufs=3, space="PSUM"))
        pv_ps = actx.enter_context(tc.tile_pool(name="pv_ps", bufs=2, space="PSUM"))
        dn_ps = actx.enter_context(tc.tile_pool(name="dn_ps", bufs=2, space="PSUM"))
        oo_ps = actx.enter_context(tc.tile_pool(name="oo_ps", bufs=1, space="PSUM"))
        wo_pool = actx.enter_context(tc.tile_pool(name="wo", bufs=1))

        woT_sb = wo_pool.tile([P, ET, DQ], F32R)
        nc.sync.dma_start(woT_sb[:], woT_in.rearrange("(et p) d -> p et d", p=P).bitcast(F32R))
        outAcc = wo_pool.tile([P, 4, S], F32)

        for h in range(NHL):
            for qc in range(NQC):
                q0 = qc * 512
                nkt = 4 * qc + 4
                qT = QT_sb[:, h, q0:q0 + 512]
                pv = pv_ps.tile([P, 512], F32, name="pv", tag="pv")
                pden = dn_ps.tile([P, 512], F32, name="pden", tag="pden")
                denom = (None if DEN_PE else
                         dn_pool.tile([P, 512], F32, name="denom", tag="denom"))
                exs = [None] * nkt
                LAG = 4  # pv(kt-LAG) emitted after scores(kt): hides exp+mask
                def emit_pv(j, last):
                    nc.tensor.matmul(pv[:], V_sb[:, j, :], exs[j][:],
                                     start=(j == 0), stop=last)
                for kt in range(nkt):
                    ps = sc_ps.tile([P, 512], F32, name="ps", tag="ps")
                    nc.tensor.matmul(
                        ps[:], KT_sb[:, kt * P:(kt + 1) * P], qT,
                        start=True, stop=True)
                    ex = ex_pool.tile([P, 512], F32R, name="ex", tag="ex")
                    exs[kt] = ex
                    nc.scalar.activation(ex[:], ps[:],
                                         mybir.ActivationFunctionType.Exp,
                                         scale=float(SCALE))
                    t = kt - 4 * qc
                    if t >= 0:
                        if GPSIMD_MASK:
                            nc.gpsimd.affine_select(
                                out=ex[:], in_=ex[:],
                                compare_op=mybir.AluOpType.is_ge,
                                fill=0.0, base=-P * t, pattern=[[1, 512]],
                                channel_multiplier=-1)
                        else:
                            nc.vector.tensor_tensor(
                                ex[:], ex[:].bitcast(F32),
                                masks[:, t * 512:(t + 1) * 512],
                                mybir.AluOpType.mult)
                    if DEN_PE:
                        nc.tensor.matmul(pden[0:1, :], ones_red[:], ex[:],
                                         start=(kt == 0),
                                         stop=(kt == nkt - 1))
                    elif kt == 0:
                        nc.vector.tensor_copy(denom[:], ex[:].bitcast(F32))
                    else:
                        nc.vector.tensor_add(denom[:], denom[:],
                                             ex[:].bitcast(F32))
                    if kt >= LAG:
                        emit_pv(kt - LAG, last=False)
                for j in range(max(0, nkt - LAG), nkt):
                    emit_pv(j, last=(j == nkt - 1))
                if not DEN_PE:
                    # denom -> f32r -> partition-reduce
                    denr = dn_pool.tile([P, 512], F32R, name="denr", tag="denr")
                    nc.vector.tensor_copy(denr[:], denom[:])
                    nc.tensor.matmul(pden[0:1, :], ones_red[:], denr[:],
                                     start=True, stop=True)
                rec = sm_pool.tile([1, 512], F32R, name="rec", tag="rec")
                nc.vector.reciprocal(rec[:], pden[0:1, :])
                pbc = dn_ps.tile([P, 512], F32, name="pbc", tag="pden")
                nc.tensor.matmul(pbc[:], ones_col[:], rec[:],
                                 start=True, stop=True)
                bcr = bc_pool.tile([P, 512], F32, name="bcr", tag="bcr")
                nc.scalar.copy(bcr[:], pbc[:])
                outH = oh_pool.tile([P, 512], F32, name="outH", tag="outH")
                nc.vector.tensor_tensor(outH[:], pv[:], bcr[:],
                                        mybir.AluOpType.mult)
                NHALF = min(AG_HALVES, NQC)
                cph = NQC // NHALF
                hf = qc // cph
                qh0 = (qc - hf * cph) * 512
                nc.sync.dma_start(agin[h][hf][:, qh0:qh0 + 512], outH[:])

            # ---- ship head h: AllGather halves across the batch group ----
            NHALF = min(AG_HALVES, NQC)
            cph = NQC // NHALF
            for hf in range(NHALF):
                if NO_COLLECTIVE:
                    for mt in range(4):
                        nc.sync.dma_start(
                            agout[h][hf][mt * P:(mt + 1) * P, :],
                            agin[h][hf][:])
                else:
                    nc.gpsimd.collective_compute(
                        "AllGather", mybir.AluOpType.bypass,
                        replica_groups=REPLICA_GROUPS,
                        ins=[agin[h][hf].opt()],
                        outs=[agout[h][hf].opt()])
            # o_proj for head h-1: delayed one head so the PE never waits
            # on this head's AllGather inside its in-order stream
            if h > 0:
                _emit_oproj(nc, h - 1, NQC, cph, NHALF, agout, af_pool,
                            oo_ps, woT_sb, outAcc, out_d)
        _emit_oproj(nc, NHL - 1, NQC, cph, NHALF, agout, af_pool,
                    oo_ps, woT_sb, outAcc, out_d)


# ======================= host side =======================

_CACHE = {}


def _get_program(S=2048, reps=1):
    key = (S, reps, AG_HALVES, GPSIMD_MASK, NO_COLLECTIVE, X_ROWS, DEN_PE)
    if key not in _CACHE:
        _CACHE[key] = build_program(S=S, reps=reps)
    return _CACHE[key]


def make_in_maps(x, cos, sin, wq, wk, wv, wo):
    in_maps = []
    cosT = np.ascontiguousarray(cos.T.astype(np.float32))
    sinT = sin.T.astype(np.float32).copy()
    sinT[:HD // 2, :] *= -1.0   # fold rotate_half sign into the table
    sinT = np.ascontiguousarray(sinT)
    for c in range(8):
        b, g = c // 4, c % 4
        in_maps.append({
            "x": np.ascontiguousarray(x[b].astype(np.float32)),
            "wqT": np.ascontiguousarray(wq[g * DQ:(g + 1) * DQ, :].T.astype(np.float32)),
            "wkT": np.ascontiguousarray(wk[g * HD:(g + 1) * HD, :].T.astype(np.float32)),
            "wvT": np.ascontiguousarray(wv[g * HD:(g + 1) * HD, :].T.astype(np.float32)),
            "woT": np.ascontiguousarray(wo[g * DQ:(g + 1) * DQ, :].T.astype(np.float32)),
            "cosT": cosT,
            "sinT": sinT,
        })
    return in_maps


def assemble_output(results, B, S):
    out = np.empty((B, S, E), np.float32)
    for c in range(8):
        b, g = c // 4, c % 4
        out[b][:, g * DQ:(g + 1) * DQ] = results[c]["out"].T
    return out


# ---- inline SPMD runner (PJRT/axon), device-resident inputs ----

class SpmdRunner:
    def __init__(self, nc, n_cores):
        import jax
        from jax.sharding import Mesh, PartitionSpec
        from jax.experimental.shard_map import shard_map
        from concourse import bass2jax
        from concourse.bass2jax import _bass_exec_p, install_neuronx_cc_hook

        install_neuronx_cc_hook()
        self.jax = jax
        self.nc = nc
        self.n_cores = n_cores
        partition_name = (nc.partition_id_tensor.name
                          if nc.partition_id_tensor else None)
        in_names, out_names, out_avals = [], [], []
        zero_outs = []
        for alloc in nc.m.functions[0].allocations:
            if not isinstance(alloc, mybir.MemoryLocationSet):
                continue
            name = alloc.memorylocations[0].name
            if alloc.kind == "ExternalInput":
                if name != partition_name:
                    in_names.append(name)
            elif alloc.kind == "ExternalOutput":
                out_names.append(name)
                shape = tuple(alloc.tensor_shape)
                dtype = mybir.dt.np(alloc.dtype)
                out_avals.append(jax.core.ShapedArray(shape, dtype))
                zero_outs.append(np.zeros(shape, dtype))
        self.in_names, self.out_names = in_names, out_names
        self.out_avals, self.zero_outs = out_avals, zero_outs
        self.n_params = len(in_names)

        all_in = list(in_names) + list(out_names)
        if partition_name is not None:
            all_in.append(partition_name)

        def _body(*args):
            operands = list(args)
            if partition_name is not None:
                operands.append(bass2jax.partition_id_tensor())
            outs = _bass_exec_p.bind(
                *operands, out_avals=tuple(out_avals),
                in_names=tuple(all_in), out_names=tuple(out_names),
                lowering_input_output_aliases=(),
                sim_require_finite=True, sim_require_nnan=True, nc=nc)
            return tuple(outs)

        devices = jax.devices()[:n_cores]
        self.mesh = Mesh(np.asarray(devices), ("core",))
        n_outs = len(out_names)
        in_specs = (PartitionSpec("core"),) * (self.n_params + n_outs)
        out_specs = (PartitionSpec("core"),) * n_outs
        self.fn = jax.jit(
            shard_map(_body, mesh=self.mesh, in_specs=in_specs,
                      out_specs=out_specs, check_rep=False),
            keep_unused=True)
        self.dev_args = None

    def stage_inputs(self, in_maps):
        import jax
        from jax.sharding import PartitionSpec
        per_core = [[np.asarray(m[n]) for n in self.in_names] for m in in_maps]
        concat_in = [
            np.concatenate([per_core[c][i] for c in range(self.n_cores)], axis=0)
            for i in range(self.n_params)]
        concat_zeros = [
            np.zeros((self.n_cores * z.shape[0], *z.shape[1:]), z.dtype)
            for z in self.zero_outs]
        sharding = jax.sharding.NamedSharding(self.mesh, PartitionSpec("core"))
        self.dev_args = [jax.device_put(a, sharding)
                         for a in (*concat_in, *concat_zeros)]
        for a in self.dev_args:
            a.block_until_ready()

    def run(self):
        out_arrs = [np.asarray(o) for o in self.fn(*self.dev_args)]
        return [
            {n: out_arrs[i].reshape(self.n_cores, *self.out_avals[i].shape)[c]
             for i, n in enumerate(self.out_names)}
            for c in range(self.n_cores)]

    def time_exec(self, iters=30, warmup=3):
        import jax
        for _ in range(warmup):
            res = self.fn(*self.dev_args)
        jax.block_until_ready(res)
        t0 = time.perf_counter()
        for _ in range(iters):
            res = self.fn(*self.dev_args)
        jax.block_until_ready(res)
        t1 = time.perf_counter()
        return (t1 - t0) / iters * 1e9


_RUNNER_CACHE = {}


def get_runner(S=2048, reps=1):
    key = (S, reps, AG_HALVES, GPSIMD_MASK, NO_COLLECTIVE, X_ROWS, DEN_PE)
    if key not in _RUNNER_CACHE:
        nc = _get_program(S=S, reps=reps)
        _RUNNER_CACHE[key] = SpmdRunner(nc, 8)
    return _RUNNER_CACHE[key]


def kernel(x, cos, sin, wq, wk, wv, wo):
    B, S, _ = x.shape
    runner = get_runner(S=S, reps=1)
    runner.stage_inputs(make_in_maps(x, cos, sin, wq, wk, wv, wo))
    results = runner.run()
    return assemble_output(results, B, S)


if __name__ == "__main__":
    # tiny self-test against a local numpy reference
    S = int(sys.argv[1]) if len(sys.argv) > 1 else 512
    rng = np.random.default_rng(0)
    B, H, HKV = 2, 16, 4
    x = rng.standard_normal((B, S, E), dtype=np.float32)
    cos = rng.random((S, HD), dtype=np.float32)
    sin = rng.random((S, HD), dtype=np.float32)
    sc = 0.02
    wq = (rng.standard_normal((H * HD, E), dtype=np.float32) * sc)
    wk = (rng.standard_normal((HKV * HD, E), dtype=np.float32) * sc)
    wv = (rng.standard_normal((HKV * HD, E), dtype=np.float32) * sc)
    wo = (rng.standard_normal((E, H * HD), dtype=np.float32) * sc)

    def ref(x, cos, sin, wq, wk, wv, wo):
        x64 = x.astype(np.float64)
        q = (x64 @ wq.T.astype(np.float64)).reshape(B, S, H, HD)
        k = (x64 @ wk.T.astype(np.float64)).reshape(B, S, HKV, HD)
        v = (x64 @ wv.T.astype(np.float64)).reshape(B, S, HKV, HD)

        def rot(t):
            return np.concatenate([-t[..., HD // 2:], t[..., :HD // 2]], -1)

        c = cos[:, None, :].astype(np.float64)
        s = sin[:, None, :].astype(np.float64)
        q = q * c + rot(q) * s
        k = k * c + rot(k) * s
        k = np.repeat(k, H // HKV, axis=2).transpose(0, 2, 1, 3)
        v = np.repeat(v, H // HKV, axis=2).transpose(0, 2, 1, 3)
        q = q.transpose(0, 2, 1, 3)
        scores = np.einsum("bhqd,bhkd->bhqk", q, k) / np.sqrt(HD)
        mask = np.tril(np.ones((S, S), bool))
        scores = np.where(mask, scores, -np.inf)
        scores -= scores.max(-1, keepdims=True)
        p = np.exp(scores)
        p /= p.sum(-1, keepdims=True)
        o = np.einsum("bhqk,bhkd->bhqd", p, v)
        o = o.transpose(0, 2, 1, 3).reshape(B, S, H * HD)
        return o @ wo.T.astype(np.float64)

    want = ref(x, cos, sin, wq, wk, wv, wo)
    got = kernel(x, cos, sin, wq, wk, wv, wo)
    err = np.abs(got - want).max() / np.abs(want).max()
    print(f"S={S}: rel err (absmax-relative) = {err:.3e}")

